# revision 25
# baseline (speedup 1.0000x reference)
"""DepthwiseXCorr (SiamRPN-style depthwise cross-correlation head) on 8 trn2 cores.

Data-parallel over batch: B=128 -> 16 samples per core. Per sample:
  branch(x) = BN2(pw1x1(ReLU6(BN1(dw3x3(x)))))   for kernel (7x7) and search (31x31)
  out = per-channel xcorr(search_feat 29x29, kernel_feat 5x5) -> 25x25

v2 strategy: move the per-channel conv FMA work (dw 3x3 and the 25-tap xcorr)
off VectorE onto TensorE as diagonal-weight matmuls, using 32x32 sub-array
tile_position concurrency to run 8 independent diag matmuls at once
(4 channel groups x 2 spatial output slices; the second slice's output lands
partition-rotated by 64, compensated by rotated pw weights / rotated out DMA).
All matmul operands are bf16 (inputs cast host-side), accumulation in fp32
PSUM; biases/evictions in fp32.

Engine roles per (sample, 128-ch block):
  - TensorE: dw convs (host-folded diag weights), pw convs, xcorr
    (on-chip-built diag weights)
  - VectorE: builds xcorr diag weights (mask * K2 column), relu6 min-clamp,
    one xcorr PSUM eviction
  - ScalarE: PSUM evictions with bias via activation (Relu / Identity)
  - DMA: bf16 inputs, fp32 outputs (un-rotating slice-1 with split DMAs)
"""

import numpy as np
import ml_dtypes

import concourse.bass as bass
import concourse.mybir as mybir
from concourse.tile import TileContext
from concourse.bass_utils import run_bass_kernel_spmd

F32 = mybir.dt.float32
F32R = mybir.dt.float32r
BF16 = mybir.dt.bfloat16
AF = mybir.ActivationFunctionType
OP = mybir.AluOpType

B, C, KH, SH, KK = 128, 256, 7, 31, 3
N_CORES = 8
BPC = B // N_CORES          # samples per core
G = C // 128                # channel blocks
EPS = 1e-5

HO_K, HO_S, HO_X = KH - 2, SH - 2, 25   # 5, 29, 25

# spatial output-row splits (second slice partition-rotated by 64)
DW_R0, DW_R1 = 15, 14       # dw-s output rows 29 = 15 + 14  (FD 435 / 406)
XC_R0, XC_R1 = 13, 12
VB_SAMP = 5                 # samples whose xcorr runs on VectorE       # xcorr output rows 25 = 13 + 12 (FD 325 / 300)
FD_DW0, FD_DW1 = DW_R0 * HO_S, DW_R1 * HO_S
FD_XC0, FD_XC1 = XC_R0 * HO_X, XC_R1 * HO_X

# bf16 params column layout
O_DK = 0                          # dw-k full diag: (g,t) -> 128 cols   [2304]
O_DS = O_DK + G * 9 * 128         # dw-s diag64:    (g,t) -> 64 cols    [1152]
O_WPK = O_DS + G * 9 * 64         # pw-k lhsT blocks (gi,go) 128 cols   [512]
O_WPS = O_WPK + 512               # pw-s lhsT blocks                    [512]
O_WPSR = O_WPS + 512              # pw-s lhsT blocks rotated -64        [512]
O_M32 = O_WPSR + 512              # 64-diag mask                        [64]
PB_TOT = O_M32 + 64
# fp32 params column layout
O_BDK = 0                         # dw-k bn1 shift (g)                  [2]
O_BDS = O_BDK + G                 # dw-s bn1 shift                      [2]
O_BDSR = O_BDS + G                # dw-s bn1 shift rotated              [2]
O_BPK = O_BDSR + G                # pw-k bias (go)                      [2]
O_BPS = O_BPK + G                 # pw-s bias                           [2]
PF_TOT = O_BPS + G

_cache: dict = {}

LAST_RESULTS = None         # stash for test harness (exec_time_ns etc.)


def _fold_branch(dw_w, bn1, pw_w, pw_b, bn2):
    """Fold eval-mode BN params into conv weights/biases (host, fp64->fp32)."""
    dw_w = dw_w.astype(np.float64)
    bn1 = bn1.astype(np.float64)
    pw_w = pw_w.astype(np.float64)
    pw_b = pw_b.astype(np.float64)
    bn2 = bn2.astype(np.float64)
    g1, b1, m1, v1 = bn1[0], bn1[1], bn1[2], bn1[3]
    inv1 = g1 / np.sqrt(v1 + EPS)
    shift1 = (b1 - m1 * inv1).astype(np.float32)
    dw = (dw_w[:, 0] * inv1[:, None, None]).reshape(C, 9).astype(np.float32)

    g2, b2, m2, v2 = bn2[0], bn2[1], bn2[2], bn2[3]
    inv2 = g2 / np.sqrt(v2 + EPS)
    W = (pw_w[:, :, 0, 0] * inv2[:, None]).astype(np.float32)   # (co, ci)
    bias2 = (pw_b * inv2 + (b2 - m2 * inv2)).astype(np.float32)

    # lhsT blocks for PE: lhsT[gi, go][ci_in, co_in] = W[go*128+co_in, gi*128+ci_in]
    lhsT = np.zeros((G, G, 128, 128), np.float32)
    for gi in range(G):
        for go in range(G):
            lhsT[gi, go] = W[go * 128:(go + 1) * 128, gi * 128:(gi + 1) * 128].T
    dw_blk = dw.reshape(G, 128, 9)
    b1_blk = shift1.reshape(G, 128)
    b2_blk = bias2.reshape(G, 128)
    return dw_blk, b1_blk, lhsT, b2_blk


def _split_waits(nc, keep=1):
    """This container's walrus accepts only one sync-wait per instruction.
    Move extra waits onto standalone EventSemaphore instructions placed just
    before the owning instruction in its engine stream (same semantics: the
    engine's sequencer stalls on each in turn)."""
    import bass_rust

    n = 0
    for bb in nc.m.functions[0].blocks:
        out = []
        for ins in bb.instructions:
            si = ins.sync_info
            if si is not None and len(si.on_wait) > keep:
                waits = list(si.on_wait)
                for w in waits[:-keep]:
                    n += 1
                    ev = mybir.InstEventSemaphore(
                        name=f"antsplitw_{n}", ins=[], outs=[])
                    ev.engine = ins.engine
                    ev.sync_info = bass_rust.SyncInfo(on_wait=[w], on_update=[])
                    out.append(ev)
                ins.sync_info = bass_rust.SyncInfo(
                    on_wait=waits[-keep:], on_update=list(si.on_update))
            out.append(ins)
        bb.instructions = out
    return n


def _build_nc():
    """Build the per-core Bass kernel (same program on all 8 cores)."""
    nc = bass.Bass()

    kern_h = nc.declare_dram_parameter("kern_in", [BPC, C, KH, KH], BF16, isOutput=False)
    srch_h = nc.declare_dram_parameter("srch_in", [BPC, C, SH, SH], BF16, isOutput=False)
    prmb_h = nc.declare_dram_parameter("paramsb", [128, PB_TOT], BF16, isOutput=False)
    prmf_h = nc.declare_dram_parameter("paramsf", [128, PF_TOT], F32, isOutput=False)
    out_h = nc.declare_dram_parameter("out", [BPC, C, HO_X, HO_X], F32, isOutput=True)

    with TileContext(nc) as tc:
        with (
            tc.tile_pool(name="const", bufs=1) as cpool,
            tc.tile_pool(name="kio", bufs=2) as kpool,
            tc.tile_pool(name="sio", bufs=4) as spool,
            tc.tile_pool(name="feat", bufs=2) as fpool,
            tc.tile_pool(name="diag", bufs=2) as dpool,
            tc.tile_pool(name="xout", bufs=2) as xpool,
            tc.tile_pool(name="pswork", bufs=2, space="PSUM") as pp1,
            tc.tile_pool(name="psx", bufs=2, space="PSUM") as ppx,
        ):
            # ---- constants into SBUF ----
            prmb = cpool.tile([128, PB_TOT], BF16)
            nc.sync.dma_start(out=prmb[:], in_=prmb_h[:])
            prmf = cpool.tile([128, PF_TOT], F32)
            nc.sync.dma_start(out=prmf[:], in_=prmf_h[:])

            def _b(base, g):          # fp32 bias col [128,1]
                return prmf[:, base + g:base + g + 1]

            def _dk(g, t):            # dw-k full diag [128,128] bf16
                o = O_DK + (g * 9 + t) * 128
                return prmb[:, o:o + 128]

            def _ds(g, t, h):         # dw-s diag64 row-block [64,64] bf16
                o = O_DS + (g * 9 + t) * 64
                return prmb[64 * h:64 * (h + 1), o:o + 64]

            def _wp(base, gi, go):    # pw lhsT block [128,128] bf16
                o = base + (gi * G + go) * 128
                return prmb[:, o:o + 128]

            # ====== kernel branch, batched across all BPC samples ======
            NKB = BPC * HO_K * HO_K     # 400 cols: [b, 5x5] per partition
            Hk = []
            for g in range(G):
                xk = kpool.tile([128, BPC, KH, KH], BF16, name="xk")
                nc.sync.dma_start(
                    out=xk[:],
                    in_=kern_h[:, 128 * g:128 * (g + 1)].rearrange(
                        "b c h w -> c b h w"))
                pk = pp1.tile([128, BPC, HO_K * HO_K], F32, name="pk", tag="pd0")
                for t in range(9):
                    u, v = t // 3, t % 3
                    nc.tensor.matmul(
                        pk[:], _dk(g, t), xk[:, :, u:u + HO_K, v:v + HO_K],
                        start=(t == 0), stop=(t == 8))
                hk = fpool.tile([128, NKB], BF16, name="hk")
                nc.scalar.activation(hk[:], pk[:].rearrange("p b n -> p (b n)"),
                                     AF.Relu, bias=_b(O_BDK, g), scale=1.0)
                nc.vector.tensor_scalar(hk[:], hk[:], 6.0, None, OP.min)
                Hk.append(hk)
            K2 = []
            K2F = []
            for go in range(G):
                pko = pp1.tile([128, NKB], F32, name="pko", tag="pd1")
                for gi in range(G):
                    nc.tensor.matmul(
                        pko[:], _wp(O_WPK, gi, go), Hk[gi][:],
                        start=(gi == 0), stop=(gi == G - 1))
                k2 = cpool.tile([128, BPC, HO_K * HO_K], BF16, name=f"k2_{go}")
                nc.scalar.activation(k2[:].rearrange("p b n -> p (b n)"), pko[:],
                                     AF.Identity, bias=_b(O_BPK, go), scale=1.0)
                k2f = cpool.tile([128, BPC, HO_K * HO_K], F32, name=f"k2f_{go}")
                nc.vector.tensor_copy(k2f[:], k2[:])
                K2.append(k2)
                K2F.append(k2f)

            for b in range(BPC):
                # ================= search branch =================
                Hs = []   # per g: (Hs0 [128, DW_R0*29] aligned, Hs1 [128, DW_R1*29] rot64)
                for g in range(G):
                    xs = spool.tile([128, SH, SH], BF16, name="xs")
                    nc.sync.dma_start(out=xs[:], in_=srch_h[b, 128 * g:128 * (g + 1)])
                    pd0 = pp1.tile([128, FD_DW0], F32, name="pd0", tag="pd0")
                    pd1 = pp1.tile([128, FD_DW1], F32, name="pd1", tag="pd1")
                    for t in range(9):
                        u, v = t // 3, t % 3
                        for h in range(2):
                            # slice 0: rows 0..DW_R0, tile (h, h)
                            nc.tensor.matmul(
                                pd0[64 * h:64 * (h + 1), :], _ds(g, t, h),
                                xs[64 * h:64 * (h + 1), u:u + DW_R0, v:v + HO_S],
                                start=(t == 0), stop=(t == 8),
                                tile_position=(64 * h, 64 * h))
                            # slice 1: rows DW_R0.., tile (h, 1-h)
                            j = 1 - h
                            nc.tensor.matmul(
                                pd1[64 * j:64 * (j + 1), :], _ds(g, t, h),
                                xs[64 * h:64 * (h + 1),
                                   u + DW_R0:u + DW_R0 + DW_R1, v:v + HO_S],
                                start=(t == 0), stop=(t == 8),
                                tile_position=(64 * h, 64 * j))
                    h0 = fpool.tile([128, FD_DW0], BF16, name="h0")
                    h1 = fpool.tile([128, FD_DW1], BF16, name="h1")
                    nc.scalar.activation(h0[:], pd0[:], AF.Relu,
                                         bias=_b(O_BDS, g), scale=1.0)
                    nc.vector.tensor_scalar(h1[:], pd1[:], _b(O_BDSR, g), 0.0,
                                            OP.add, OP.max)
                    nc.vector.tensor_scalar(h0[:], h0[:], 6.0, None, OP.min)
                    nc.vector.tensor_scalar(h1[:], h1[:], 6.0, None, OP.min)
                    Hs.append((h0, h1))
                S2 = []
                for go in range(G):
                    s2 = fpool.tile([128, HO_S, HO_S], BF16, name="s2")
                    s2f = s2[:].rearrange("p a b -> p (a b)")
                    ps0 = pp1.tile([128, FD_DW0], F32, name="ps0", tag="pd0")
                    ps1 = pp1.tile([128, FD_DW1], F32, name="ps1", tag="pd1")
                    for gi in range(G):
                        nc.tensor.matmul(
                            ps0[:], _wp(O_WPS, gi, go), Hs[gi][0][:],
                            start=(gi == 0), stop=(gi == G - 1))
                    for gi in range(G):
                        nc.tensor.matmul(
                            ps1[:], _wp(O_WPSR, gi, go), Hs[gi][1][:],
                            start=(gi == 0), stop=(gi == G - 1))
                    nc.scalar.activation(s2f[:, 0:FD_DW0], ps0[:], AF.Identity,
                                         bias=_b(O_BPS, go), scale=1.0)
                    nc.vector.tensor_scalar(s2f[:, FD_DW0:FD_DW0 + FD_DW1], ps1[:],
                                            _b(O_BPS, go), None, OP.add)
                    S2.append(s2)

                # ================= depthwise xcorr =================
                # PE path: 64x64 diag matmuls, 4-way sub-array concurrency.
                # V path (last VB_SAMP samples): VectorE 25-tap STT in bf16,
                # using a one-col-shifted S2 copy so every tap hits 2x mode.
                for g in range(G):
                    s2 = S2[g]
                    if b >= BPC - VB_SAMP:
                        s2s = dpool.tile([128, HO_S, HO_S - 1], BF16, name="s2s")
                        nc.vector.tensor_copy(s2s[:], s2[:, :, 1:HO_S])
                        acc = xpool.tile([128, HO_X, HO_X], BF16, name="acc",
                                         tag="accb")
                        k2c = K2F[g]
                        for t in range(HO_X):
                            u, v = t // 5, t % 5
                            if v % 2 == 0:
                                win = s2[:, u:u + HO_X, v:v + HO_X]
                            else:
                                win = s2s[:, u:u + HO_X, v - 1:v - 1 + HO_X]
                            kcol = k2c[:, b, t:t + 1]
                            if t == 0:
                                nc.vector.tensor_scalar(
                                    acc[:], win, kcol, None, OP.mult)
                            else:
                                nc.vector.scalar_tensor_tensor(
                                    acc[:], win, kcol, acc[:], OP.mult, OP.add)
                        accf = xpool.tile([128, HO_X, HO_X], F32, name="accf",
                                          tag="accf")
                        nc.vector.tensor_copy(accf[:], acc[:])
                        nc.sync.dma_start(
                            out=out_h[b, 128 * g:128 * (g + 1)], in_=accf[:])
                        continue

                    dall = dpool.tile([128, HO_X, 64], BF16, name="dall")
                    k2ap, m64ap = K2[g][:, b:b + 1, :], prmb[:, O_M32:O_M32 + 64]
                    k2b = bass.AP(k2ap.tensor, k2ap.offset,
                                  [k2ap.ap[0], (1, HO_X), (0, 64)])
                    m64b = bass.AP(m64ap.tensor, m64ap.offset,
                                   [m64ap.ap[0], (0, HO_X), (1, 64)])
                    nc.vector.tensor_tensor(dall[:], k2b, m64b, OP.mult)

                    px0 = ppx.tile([128, FD_XC0], F32, name="px0")
                    px1 = ppx.tile([128, FD_XC1], F32, name="px1")
                    for t in range(HO_X):
                        u, v = t // 5, t % 5
                        for h in range(2):
                            nc.tensor.matmul(
                                px0[64 * h:64 * (h + 1), :],
                                dall[64 * h:64 * (h + 1), t:t + 1, :],
                                s2[64 * h:64 * (h + 1), u:u + XC_R0, v:v + HO_X],
                                start=(t == 0), stop=(t == HO_X - 1),
                                tile_position=(64 * h, 64 * h))
                            j = 1 - h
                            nc.tensor.matmul(
                                px1[64 * j:64 * (j + 1), :],
                                dall[64 * h:64 * (h + 1), t:t + 1, :],
                                s2[64 * h:64 * (h + 1),
                                   u + XC_R0:u + XC_R0 + XC_R1, v:v + HO_X],
                                start=(t == 0), stop=(t == HO_X - 1),
                                tile_position=(64 * h, 64 * j))
                    xo0 = xpool.tile([128, FD_XC0], F32, name="xo0")
                    xo1 = xpool.tile([128, FD_XC1], F32, name="xo1")
                    nc.scalar.activation(xo0[:], px0[:], AF.Identity, scale=1.0)
                    nc.vector.tensor_copy(xo1[:], px1[:])
                    # slice 0: aligned channels
                    nc.sync.dma_start(
                        out=out_h[b, 128 * g:128 * (g + 1), 0:XC_R0, :],
                        in_=xo0[:].rearrange("p (a b) -> p a b", a=XC_R0))
                    # slice 1: partition p holds channel (p+64)%128
                    nc.sync.dma_start(
                        out=out_h[b, 128 * g + 64:128 * (g + 1), XC_R0:HO_X, :],
                        in_=xo1[0:64, :].rearrange("p (a b) -> p a b", a=XC_R1))
                    nc.sync.dma_start(
                        out=out_h[b, 128 * g:128 * g + 64, XC_R0:HO_X, :],
                        in_=xo1[64:128, :].rearrange("p (a b) -> p a b", a=XC_R1))
    _split_waits(nc)
    return nc


def _pack_params(kdw, kb1, kpw, kb2, sdw, sb1, spw, sb2):
    prmb = np.zeros((128, PB_TOT), np.float32)
    prmf = np.zeros((128, PF_TOT), np.float32)
    p = np.arange(128)
    # dw-k full diags
    for g in range(G):
        for t in range(9):
            o = O_DK + (g * 9 + t) * 128
            prmb[p, o + p] = kdw[g, :, t]
    # dw-s diag64
    for g in range(G):
        for t in range(9):
            o = O_DS + (g * 9 + t) * 64
            prmb[p, o + (p % 64)] = sdw[g, :, t]
    # pw weights
    for gi in range(G):
        for go in range(G):
            o = O_WPK + (gi * G + go) * 128
            prmb[:, o:o + 128] = kpw[gi, go]
            o = O_WPS + (gi * G + go) * 128
            prmb[:, o:o + 128] = spw[gi, go]
            o = O_WPSR + (gi * G + go) * 128
            prmb[:, o:o + 128] = np.roll(spw[gi, go], -64, axis=0)
    prmb[p, O_M32 + (p % 64)] = 1.0
    for g in range(G):
        prmf[:, O_BDK + g] = kb1[g]
        prmf[:, O_BDS + g] = sb1[g]
        prmf[:, O_BDSR + g] = np.roll(sb1[g], -64)
        prmf[:, O_BPK + g] = kb2[g]
        prmf[:, O_BPS + g] = sb2[g]
    return prmb.astype(ml_dtypes.bfloat16), prmf


def kernel(kernel, search, k_dw_w, k_bn1, k_pw_w, k_pw_b, k_bn2,
           s_dw_w, s_bn1, s_pw_w, s_pw_b, s_bn2):
    global LAST_RESULTS
    kdw, kb1, kpw, kb2 = _fold_branch(np.asarray(k_dw_w), np.asarray(k_bn1),
                                      np.asarray(k_pw_w), np.asarray(k_pw_b),
                                      np.asarray(k_bn2))
    sdw, sb1, spw, sb2 = _fold_branch(np.asarray(s_dw_w), np.asarray(s_bn1),
                                      np.asarray(s_pw_w), np.asarray(s_pw_b),
                                      np.asarray(s_bn2))
    kern = np.ascontiguousarray(
        np.asarray(kernel, np.float32).astype(ml_dtypes.bfloat16))
    srch = np.ascontiguousarray(
        np.asarray(search, np.float32).astype(ml_dtypes.bfloat16))

    if "nc" not in _cache:
        _cache["nc"] = _build_nc()
    nc = _cache["nc"]

    prmb, prmf = _pack_params(kdw, kb1, kpw, kb2, sdw, sb1, spw, sb2)

    in_maps = []
    for i in range(N_CORES):
        sl = slice(i * BPC, (i + 1) * BPC)
        in_maps.append({"kern_in": kern[sl], "srch_in": srch[sl],
                        "paramsb": prmb, "paramsf": prmf})

    res = run_bass_kernel_spmd(nc, in_maps, list(range(N_CORES)))
    LAST_RESULTS = res
    out = np.concatenate([res.results[i]["out"] for i in range(N_CORES)], axis=0)
    return out


# revision 26
# speedup vs baseline: 1.1972x; 1.1972x over previous
"""DepthwiseXCorr (SiamRPN-style depthwise cross-correlation head) on 8 trn2 cores.

Data-parallel over batch: B=128 -> 16 samples per core. Per sample:
  branch(x) = BN2(pw1x1(ReLU6(BN1(dw3x3(x)))))   for kernel (7x7) and search (31x31)
  out = per-channel xcorr(search_feat 29x29, kernel_feat 5x5) -> 25x25

v2 strategy: move the per-channel conv FMA work (dw 3x3 and the 25-tap xcorr)
off VectorE onto TensorE as diagonal-weight matmuls, using 32x32 sub-array
tile_position concurrency to run 8 independent diag matmuls at once
(4 channel groups x 2 spatial output slices; the second slice's output lands
partition-rotated by 64, compensated by rotated pw weights / rotated out DMA).
All matmul operands are bf16 (inputs cast host-side), accumulation in fp32
PSUM; biases/evictions in fp32.

Engine roles per (sample, 128-ch block):
  - TensorE: dw convs (host-folded diag weights), pw convs, xcorr
    (on-chip-built diag weights)
  - VectorE: builds xcorr diag weights (mask * K2 column), relu6 min-clamp,
    one xcorr PSUM eviction
  - ScalarE: PSUM evictions with bias via activation (Relu / Identity)
  - DMA: bf16 inputs, fp32 outputs (un-rotating slice-1 with split DMAs)
"""

import numpy as np
import ml_dtypes

import concourse.bass as bass
import concourse.mybir as mybir
from concourse.tile import TileContext
from concourse.bass_utils import run_bass_kernel_spmd

F32 = mybir.dt.float32
F32R = mybir.dt.float32r
BF16 = mybir.dt.bfloat16
AF = mybir.ActivationFunctionType
OP = mybir.AluOpType

B, C, KH, SH, KK = 128, 256, 7, 31, 3
N_CORES = 8
BPC = B // N_CORES          # samples per core
G = C // 128                # channel blocks
EPS = 1e-5

HO_K, HO_S, HO_X = KH - 2, SH - 2, 25   # 5, 29, 25

# spatial output-row splits (second slice partition-rotated by 64)
DW_R0, DW_R1 = 15, 14       # dw-s output rows 29 = 15 + 14  (FD 435 / 406)
XC_R0, XC_R1 = 13, 12
VB_SAMP = 5                 # samples whose xcorr runs on VectorE       # xcorr output rows 25 = 13 + 12 (FD 325 / 300)
FD_DW0, FD_DW1 = DW_R0 * HO_S, DW_R1 * HO_S
FD_XC0, FD_XC1 = XC_R0 * HO_X, XC_R1 * HO_X

# bf16 params column layout
O_DK = 0                          # dw-k full diag: (g,t) -> 128 cols   [2304]
O_DS = O_DK + G * 9 * 128         # dw-s diag64:    (g,t) -> 64 cols    [1152]
O_WPK = O_DS + G * 9 * 64         # pw-k lhsT blocks (gi,go) 128 cols   [512]
O_WPS = O_WPK + 512               # pw-s lhsT blocks                    [512]
O_WPSR = O_WPS + 512              # pw-s lhsT blocks rotated -64        [512]
O_M32 = O_WPSR + 512              # 64-diag mask                        [64]
PB_TOT = O_M32 + 64
# fp32 params column layout
O_BDK = 0                         # dw-k bn1 shift (g)                  [2]
O_BDS = O_BDK + G                 # dw-s bn1 shift                      [2]
O_BDSR = O_BDS + G                # dw-s bn1 shift rotated              [2]
O_BPK = O_BDSR + G                # pw-k bias (go)                      [2]
O_BPS = O_BPK + G                 # pw-s bias                           [2]
PF_TOT = O_BPS + G

_cache: dict = {}

LAST_RESULTS = None         # stash for test harness (exec_time_ns etc.)


def _fold_branch(dw_w, bn1, pw_w, pw_b, bn2):
    """Fold eval-mode BN params into conv weights/biases (host, fp64->fp32)."""
    dw_w = dw_w.astype(np.float64)
    bn1 = bn1.astype(np.float64)
    pw_w = pw_w.astype(np.float64)
    pw_b = pw_b.astype(np.float64)
    bn2 = bn2.astype(np.float64)
    g1, b1, m1, v1 = bn1[0], bn1[1], bn1[2], bn1[3]
    inv1 = g1 / np.sqrt(v1 + EPS)
    shift1 = (b1 - m1 * inv1).astype(np.float32)
    dw = (dw_w[:, 0] * inv1[:, None, None]).reshape(C, 9).astype(np.float32)

    g2, b2, m2, v2 = bn2[0], bn2[1], bn2[2], bn2[3]
    inv2 = g2 / np.sqrt(v2 + EPS)
    W = (pw_w[:, :, 0, 0] * inv2[:, None]).astype(np.float32)   # (co, ci)
    bias2 = (pw_b * inv2 + (b2 - m2 * inv2)).astype(np.float32)

    # lhsT blocks for PE: lhsT[gi, go][ci_in, co_in] = W[go*128+co_in, gi*128+ci_in]
    lhsT = np.zeros((G, G, 128, 128), np.float32)
    for gi in range(G):
        for go in range(G):
            lhsT[gi, go] = W[go * 128:(go + 1) * 128, gi * 128:(gi + 1) * 128].T
    dw_blk = dw.reshape(G, 128, 9)
    b1_blk = shift1.reshape(G, 128)
    b2_blk = bias2.reshape(G, 128)
    return dw_blk, b1_blk, lhsT, b2_blk


def _split_waits(nc, keep=1):
    """This container's walrus accepts only one sync-wait per instruction.
    Move extra waits onto standalone EventSemaphore instructions placed just
    before the owning instruction in its engine stream (same semantics: the
    engine's sequencer stalls on each in turn)."""
    import bass_rust

    n = 0
    for bb in nc.m.functions[0].blocks:
        out = []
        for ins in bb.instructions:
            si = ins.sync_info
            if si is not None and len(si.on_wait) > keep:
                waits = list(si.on_wait)
                for w in waits[:-keep]:
                    n += 1
                    ev = mybir.InstEventSemaphore(
                        name=f"antsplitw_{n}", ins=[], outs=[])
                    ev.engine = ins.engine
                    ev.sync_info = bass_rust.SyncInfo(on_wait=[w], on_update=[])
                    out.append(ev)
                ins.sync_info = bass_rust.SyncInfo(
                    on_wait=waits[-keep:], on_update=list(si.on_update))
            out.append(ins)
        bb.instructions = out
    return n


def _build_nc():
    """Build the per-core Bass kernel (same program on all 8 cores)."""
    nc = bass.Bass()

    kern_h = nc.declare_dram_parameter("kern_in", [BPC, C, KH, KH], BF16, isOutput=False)
    srch_h = nc.declare_dram_parameter("srch_in", [BPC, C, SH, SH], BF16, isOutput=False)
    prmb_h = nc.declare_dram_parameter("paramsb", [128, PB_TOT], BF16, isOutput=False)
    prmf_h = nc.declare_dram_parameter("paramsf", [128, PF_TOT], F32, isOutput=False)
    out_h = nc.declare_dram_parameter("out", [BPC, C, HO_X, HO_X], F32, isOutput=True)

    with TileContext(nc) as tc:
        with (
            tc.tile_pool(name="const", bufs=1) as cpool,
            tc.tile_pool(name="kio", bufs=2) as kpool,
            tc.tile_pool(name="sio", bufs=2) as spool,
            tc.tile_pool(name="feat", bufs=2) as fpool,
            tc.tile_pool(name="diag", bufs=2) as dpool,
            tc.tile_pool(name="xout", bufs=2) as xpool,
            tc.tile_pool(name="pswork", bufs=2, space="PSUM") as pp1,
            tc.tile_pool(name="psx", bufs=2, space="PSUM") as ppx,
        ):
            # ---- constants into SBUF ----
            prmb = cpool.tile([128, PB_TOT], BF16)
            nc.sync.dma_start(out=prmb[:], in_=prmb_h[:])
            prmf = cpool.tile([128, PF_TOT], F32)
            nc.sync.dma_start(out=prmf[:], in_=prmf_h[:])

            def _b(base, g):          # fp32 bias col [128,1]
                return prmf[:, base + g:base + g + 1]

            def _dk(g, t):            # dw-k full diag [128,128] bf16
                o = O_DK + (g * 9 + t) * 128
                return prmb[:, o:o + 128]

            def _ds(g, t, h):         # dw-s diag64 row-block [64,64] bf16
                o = O_DS + (g * 9 + t) * 64
                return prmb[64 * h:64 * (h + 1), o:o + 64]

            def _wp(base, gi, go):    # pw lhsT block [128,128] bf16
                o = base + (gi * G + go) * 128
                return prmb[:, o:o + 128]

            # ====== kernel branch, batched across all BPC samples ======
            NKB = BPC * HO_K * HO_K     # 400 cols: [b, 5x5] per partition
            Hk = []
            for g in range(G):
                xk = kpool.tile([128, BPC, KH, KH], BF16, name="xk")
                nc.sync.dma_start(
                    out=xk[:],
                    in_=kern_h[:, 128 * g:128 * (g + 1)].rearrange(
                        "b c h w -> c b h w"))
                pk = pp1.tile([128, BPC, HO_K * HO_K], F32, name="pk", tag="pd0")
                for t in range(9):
                    u, v = t // 3, t % 3
                    nc.tensor.matmul(
                        pk[:], _dk(g, t), xk[:, :, u:u + HO_K, v:v + HO_K],
                        start=(t == 0), stop=(t == 8))
                hk = fpool.tile([128, NKB], BF16, name="hk")
                nc.scalar.activation(hk[:], pk[:].rearrange("p b n -> p (b n)"),
                                     AF.Relu, bias=_b(O_BDK, g), scale=1.0)
                nc.vector.tensor_scalar(hk[:], hk[:], 6.0, None, OP.min)
                Hk.append(hk)
            K2 = []
            K2F = []
            for go in range(G):
                pko = pp1.tile([128, NKB], F32, name="pko", tag="pd1")
                for gi in range(G):
                    nc.tensor.matmul(
                        pko[:], _wp(O_WPK, gi, go), Hk[gi][:],
                        start=(gi == 0), stop=(gi == G - 1))
                k2 = cpool.tile([128, BPC, HO_K * HO_K], BF16, name=f"k2_{go}")
                nc.scalar.activation(k2[:].rearrange("p b n -> p (b n)"), pko[:],
                                     AF.Identity, bias=_b(O_BPK, go), scale=1.0)
                k2f = cpool.tile([128, BPC, HO_K * HO_K], F32, name=f"k2f_{go}")
                nc.vector.tensor_copy(k2f[:], k2[:])
                K2.append(k2)
                K2F.append(k2f)

            for b in range(BPC):
                # ================= search branch =================
                Hs = []   # per g: (Hs0 [128, DW_R0*29] aligned, Hs1 [128, DW_R1*29] rot64)
                for g in range(G):
                    xs = spool.tile([128, SH, SH], BF16, name="xs")
                    nc.sync.dma_start(out=xs[:], in_=srch_h[b, 128 * g:128 * (g + 1)])
                    pd0 = pp1.tile([128, FD_DW0], F32, name="pd0", tag="pd0")
                    pd1 = pp1.tile([128, FD_DW1], F32, name="pd1", tag="pd1")
                    for t in range(9):
                        u, v = t // 3, t % 3
                        for h in range(2):
                            # slice 0: rows 0..DW_R0, tile (h, h)
                            nc.tensor.matmul(
                                pd0[64 * h:64 * (h + 1), :], _ds(g, t, h),
                                xs[64 * h:64 * (h + 1), u:u + DW_R0, v:v + HO_S],
                                start=(t == 0), stop=(t == 8),
                                tile_position=(64 * h, 64 * h))
                            # slice 1: rows DW_R0.., tile (h, 1-h)
                            j = 1 - h
                            nc.tensor.matmul(
                                pd1[64 * j:64 * (j + 1), :], _ds(g, t, h),
                                xs[64 * h:64 * (h + 1),
                                   u + DW_R0:u + DW_R0 + DW_R1, v:v + HO_S],
                                start=(t == 0), stop=(t == 8),
                                tile_position=(64 * h, 64 * j))
                    h0 = fpool.tile([128, FD_DW0], BF16, name="h0")
                    h1 = fpool.tile([128, FD_DW1], BF16, name="h1")
                    nc.scalar.activation(h0[:], pd0[:], AF.Relu,
                                         bias=_b(O_BDS, g), scale=1.0)
                    nc.vector.tensor_scalar(h1[:], pd1[:], _b(O_BDSR, g), 0.0,
                                            OP.add, OP.max)
                    nc.vector.tensor_scalar(h0[:], h0[:], 6.0, None, OP.min)
                    nc.vector.tensor_scalar(h1[:], h1[:], 6.0, None, OP.min)
                    Hs.append((h0, h1))
                S2 = []
                for go in range(G):
                    s2 = fpool.tile([128, HO_S, HO_S], BF16, name="s2")
                    s2f = s2[:].rearrange("p a b -> p (a b)")
                    ps0 = pp1.tile([128, FD_DW0], F32, name="ps0", tag="pd0")
                    ps1 = pp1.tile([128, FD_DW1], F32, name="ps1", tag="pd1")
                    for gi in range(G):
                        nc.tensor.matmul(
                            ps0[:], _wp(O_WPS, gi, go), Hs[gi][0][:],
                            start=(gi == 0), stop=(gi == G - 1))
                    for gi in range(G):
                        nc.tensor.matmul(
                            ps1[:], _wp(O_WPSR, gi, go), Hs[gi][1][:],
                            start=(gi == 0), stop=(gi == G - 1))
                    nc.scalar.activation(s2f[:, 0:FD_DW0], ps0[:], AF.Identity,
                                         bias=_b(O_BPS, go), scale=1.0)
                    nc.vector.tensor_scalar(s2f[:, FD_DW0:FD_DW0 + FD_DW1], ps1[:],
                                            _b(O_BPS, go), None, OP.add)
                    S2.append(s2)

                # ================= depthwise xcorr =================
                # PE path: 64x64 diag matmuls, 4-way sub-array concurrency.
                # V path (last VB_SAMP samples): VectorE 25-tap STT in bf16,
                # using a one-col-shifted S2 copy so every tap hits 2x mode.
                for g in range(G):
                    s2 = S2[g]
                    if b >= BPC - VB_SAMP:
                        s2s = dpool.tile([128, HO_S, HO_S - 1], BF16, name="s2s")
                        nc.vector.tensor_copy(s2s[:], s2[:, :, 1:HO_S])
                        acc = xpool.tile([128, HO_X, HO_X], BF16, name="acc",
                                         tag="accb")
                        k2c = K2F[g]
                        for t in range(HO_X):
                            u, v = t // 5, t % 5
                            if v % 2 == 0:
                                win = s2[:, u:u + HO_X, v:v + HO_X]
                            else:
                                win = s2s[:, u:u + HO_X, v - 1:v - 1 + HO_X]
                            kcol = k2c[:, b, t:t + 1]
                            if t == 0:
                                nc.vector.tensor_scalar(
                                    acc[:], win, kcol, None, OP.mult)
                            else:
                                nc.vector.scalar_tensor_tensor(
                                    acc[:], win, kcol, acc[:], OP.mult, OP.add)
                        accf = xpool.tile([128, HO_X, HO_X], F32, name="accf",
                                          tag="accf")
                        nc.vector.tensor_copy(accf[:], acc[:])
                        nc.sync.dma_start(
                            out=out_h[b, 128 * g:128 * (g + 1)], in_=accf[:])
                        continue

                    dall = dpool.tile([128, HO_X, 64], BF16, name="dall")
                    k2ap, m64ap = K2[g][:, b:b + 1, :], prmb[:, O_M32:O_M32 + 64]
                    k2b = bass.AP(k2ap.tensor, k2ap.offset,
                                  [k2ap.ap[0], (1, HO_X), (0, 64)])
                    m64b = bass.AP(m64ap.tensor, m64ap.offset,
                                   [m64ap.ap[0], (0, HO_X), (1, 64)])
                    nc.vector.tensor_tensor(dall[:], k2b, m64b, OP.mult)

                    px0 = ppx.tile([128, FD_XC0], F32, name="px0")
                    px1 = ppx.tile([128, FD_XC1], F32, name="px1")
                    for t in range(HO_X):
                        u, v = t // 5, t % 5
                        for h in range(2):
                            nc.tensor.matmul(
                                px0[64 * h:64 * (h + 1), :],
                                dall[64 * h:64 * (h + 1), t:t + 1, :],
                                s2[64 * h:64 * (h + 1), u:u + XC_R0, v:v + HO_X],
                                start=(t == 0), stop=(t == HO_X - 1),
                                tile_position=(64 * h, 64 * h))
                            j = 1 - h
                            nc.tensor.matmul(
                                px1[64 * j:64 * (j + 1), :],
                                dall[64 * h:64 * (h + 1), t:t + 1, :],
                                s2[64 * h:64 * (h + 1),
                                   u + XC_R0:u + XC_R0 + XC_R1, v:v + HO_X],
                                start=(t == 0), stop=(t == HO_X - 1),
                                tile_position=(64 * h, 64 * j))
                    xo0 = xpool.tile([128, FD_XC0], F32, name="xo0")
                    xo1 = xpool.tile([128, FD_XC1], F32, name="xo1")
                    nc.scalar.activation(xo0[:], px0[:], AF.Identity, scale=1.0)
                    nc.vector.tensor_copy(xo1[:], px1[:])
                    # slice 0: aligned channels
                    nc.sync.dma_start(
                        out=out_h[b, 128 * g:128 * (g + 1), 0:XC_R0, :],
                        in_=xo0[:].rearrange("p (a b) -> p a b", a=XC_R0))
                    # slice 1: partition p holds channel (p+64)%128
                    nc.sync.dma_start(
                        out=out_h[b, 128 * g + 64:128 * (g + 1), XC_R0:HO_X, :],
                        in_=xo1[0:64, :].rearrange("p (a b) -> p a b", a=XC_R1))
                    nc.sync.dma_start(
                        out=out_h[b, 128 * g:128 * g + 64, XC_R0:HO_X, :],
                        in_=xo1[64:128, :].rearrange("p (a b) -> p a b", a=XC_R1))
    _split_waits(nc)
    return nc


def _pack_params(kdw, kb1, kpw, kb2, sdw, sb1, spw, sb2):
    prmb = np.zeros((128, PB_TOT), np.float32)
    prmf = np.zeros((128, PF_TOT), np.float32)
    p = np.arange(128)
    # dw-k full diags
    for g in range(G):
        for t in range(9):
            o = O_DK + (g * 9 + t) * 128
            prmb[p, o + p] = kdw[g, :, t]
    # dw-s diag64
    for g in range(G):
        for t in range(9):
            o = O_DS + (g * 9 + t) * 64
            prmb[p, o + (p % 64)] = sdw[g, :, t]
    # pw weights
    for gi in range(G):
        for go in range(G):
            o = O_WPK + (gi * G + go) * 128
            prmb[:, o:o + 128] = kpw[gi, go]
            o = O_WPS + (gi * G + go) * 128
            prmb[:, o:o + 128] = spw[gi, go]
            o = O_WPSR + (gi * G + go) * 128
            prmb[:, o:o + 128] = np.roll(spw[gi, go], -64, axis=0)
    prmb[p, O_M32 + (p % 64)] = 1.0
    for g in range(G):
        prmf[:, O_BDK + g] = kb1[g]
        prmf[:, O_BDS + g] = sb1[g]
        prmf[:, O_BDSR + g] = np.roll(sb1[g], -64)
        prmf[:, O_BPK + g] = kb2[g]
        prmf[:, O_BPS + g] = sb2[g]
    return prmb.astype(ml_dtypes.bfloat16), prmf


def kernel(kernel, search, k_dw_w, k_bn1, k_pw_w, k_pw_b, k_bn2,
           s_dw_w, s_bn1, s_pw_w, s_pw_b, s_bn2):
    global LAST_RESULTS
    kdw, kb1, kpw, kb2 = _fold_branch(np.asarray(k_dw_w), np.asarray(k_bn1),
                                      np.asarray(k_pw_w), np.asarray(k_pw_b),
                                      np.asarray(k_bn2))
    sdw, sb1, spw, sb2 = _fold_branch(np.asarray(s_dw_w), np.asarray(s_bn1),
                                      np.asarray(s_pw_w), np.asarray(s_pw_b),
                                      np.asarray(s_bn2))
    kern = np.ascontiguousarray(
        np.asarray(kernel, np.float32).astype(ml_dtypes.bfloat16))
    srch = np.ascontiguousarray(
        np.asarray(search, np.float32).astype(ml_dtypes.bfloat16))

    if "nc" not in _cache:
        _cache["nc"] = _build_nc()
    nc = _cache["nc"]

    prmb, prmf = _pack_params(kdw, kb1, kpw, kb2, sdw, sb1, spw, sb2)

    in_maps = []
    for i in range(N_CORES):
        sl = slice(i * BPC, (i + 1) * BPC)
        in_maps.append({"kern_in": kern[sl], "srch_in": srch[sl],
                        "paramsb": prmb, "paramsf": prmf})

    res = run_bass_kernel_spmd(nc, in_maps, list(range(N_CORES)))
    LAST_RESULTS = res
    out = np.concatenate([res.results[i]["out"] for i in range(N_CORES)], axis=0)
    return out
